# revision 24
# baseline (speedup 1.0000x reference)
"""Trainium2 Bass kernel for nn_MOLELinear (MoE-style mixed linear layer).

Math (per graph g):
    mixed_w[g] = sum_e coefficients[g, e] * weight_experts[e] + weight_shared[0]
    mixed_b[g] = coefficients[g] @ bias_experts + bias_shared[0]
    out[g]     = x[g] @ mixed_w[g].T + mixed_b[g]

Strategy (8 NeuronCores, data-parallel over graphs; 8 graphs per core):
  * Beta fold (host): each core handles exactly E=8 graphs, so
    beta = C_core^-1 @ 1 exists and V_e = W_e + beta_e * W_sh satisfies
    sum_e c_ge V_e == sum_e c_ge W_e + W_sh exactly. The shared expert
    (weights AND bias) thus vanishes from the device kernel.
  * Bias rows are mixed on the host (c_g @ bias'_e, ~1 MFLOP), uploaded
    pre-replicated, and loaded over the idle gpsimd queue; they are only
    consumed by the main-phase PSUM evacuation adds.
  * MIX phase on the PE: one matmul per (o-16-group, i-512-chunk) with a
    block-diagonal coefficient matrix S1[(e,t),(g,t')] = c[g,e]*eye16 computes
    mixed rows for all 8 graphs at once (K=128 fully used; V streamed once).
    The "(g,t)-scrambled" result is unscrambled by 8 PE transpose-mode
    passes packed into a single PSUM bank (start flag only on the first --
    start=True marks the whole 2KB zero-region pending-zero), then one bulk
    strided ACT copy per o-group lands it in [i, (ib,g,o)] layout.
    The mix is software-pipelined (transposes lag the s1 matmuls by
    MIX_LAG o-groups, psA 4 deep) so the PE never stalls on the evacuation.
  * MAIN phase: x is loaded pre-transposed via DMA xbar transpose (fp16),
    double-buffered across graphs (graph 0 prefetched on the scalar queue
    during the mix); out[g] tiles accumulate over 8 i-blocks in PSUM; bias
    is added during the PSUM->SBUF evacuation (DVE tensor_tensor) into
    [128, 2048] staging tiles so stores ship as 512 KB DMAs (amortizes the
    ~0.6us issue + ~2us HBM write-receipt per DMA).
    Output is stored fp16 and widened to fp32 on the host (rel-err budget
    allows it; halves the store traffic and evacuation cost).
  * All matmul operands are fp16 (PSUM accumulation fp32).
"""

import numpy as np

import concourse.bacc as bacc
import concourse.mybir as mybir
import concourse.tile as tile
from concourse.bass_utils import run_bass_kernel_spmd

f32 = mybir.dt.float32
f16 = mybir.dt.float16  # fp16: same PE rate as bf16, 11-bit mantissa

NCORES = 8
G = 64                  # total graphs
GPC = G // NCORES       # graphs per core
R = 1024                # rows per graph
IN_F = 1024
OUT_F = 1024
E = 8                   # routed experts
NOG = OUT_F // 16       # number of 16-row o-groups (64)
NIB = IN_F // 128       # i blocks (8)
NRB = R // 128          # row blocks per graph (8)
MIX_LAG = 2             # o-groups the unscramble lags behind the s1 matmuls

_CACHED = {}


def build_kernel():
    nc = bacc.Bacc(None, target_bir_lowering=False)

    x_ext = nc.declare_dram_parameter("x", [GPC * R, IN_F], f16, isOutput=False)
    wp_ext = nc.declare_dram_parameter("wp", [NOG, 128, IN_F], f16, isOutput=False)
    wp4_ext = nc.declare_dram_parameter("wp4", [128, 2 * IN_F], f16,
                                        isOutput=False)
    s1_ext = nc.declare_dram_parameter("s1", [128, 128], f16, isOutput=False)
    id_ext = nc.declare_dram_parameter("ident", [128, 128], f16, isOutput=False)
    brep_ext = nc.declare_dram_parameter("brep", [GPC * 128, OUT_F], f16,
                                         isOutput=False)
    out_ext = nc.declare_dram_parameter("out", [GPC * R, OUT_F], f16,
                                        isOutput=True)

    with tile.TileContext(nc) as tc:
        with (
            tc.tile_pool(name="consts", bufs=1) as cpool,
            tc.tile_pool(name="mixed", bufs=1) as mpool,
            tc.tile_pool(name="breps", bufs=1) as bpool,
            tc.tile_pool(name="xtp", bufs=2) as xtpool,
            tc.tile_pool(name="wstage", bufs=4) as wpool,
            tc.tile_pool(name="scr", bufs=MIX_LAG + 1) as scrpool,
            tc.tile_pool(name="outs", bufs=3) as opool,
            tc.tile_pool(name="psA", bufs=3, space="PSUM") as psA,
            tc.tile_pool(name="psB", bufs=3, space="PSUM") as psB,
            tc.tile_pool(name="psC", bufs=2, space="PSUM") as psC,
        ):
            # ---- constants ----
            s1_t = cpool.tile([128, 128], f16, tag="s1")
            id_t = cpool.tile([128, 128], f16, tag="id")
            nc.sync.dma_start(out=s1_t[:], in_=s1_ext[:])
            nc.sync.dma_start(out=id_t[:], in_=id_ext[:])

            # ---- mixed weights, transposed: [128 i, (ib, g, o)] fp16 ----
            mixedbuf = mpool.tile([128, NIB * GPC * OUT_F], f16, tag="mixed",
                                  name="mixedbuf")
            mixedv = mixedbuf[:].rearrange("p (ib g o) -> p ib g o",
                                           ib=NIB, g=GPC)

            # ---- bias rows: host-mixed and pre-replicated, loaded on the
            # otherwise-idle gpsimd queue (keeps the PE/ACT/sync queues
            # clear so the s1 stream starts as soon as w(0) lands) ----
            breps = []
            for g in range(GPC):
                brep_t = bpool.tile([128, OUT_F], f16, tag=f"brep{g}",
                                    name=f"brep{g}")
                nc.gpsimd.dma_start(out=brep_t[:],
                                    in_=brep_ext[g * 128:(g + 1) * 128, :])
                breps.append(brep_t)

            # ---- MIX phase ----
            def emit_unscramble(og, scr_t):
                # 8 PE transposes into one PSUM bank, then one bulk copy.
                un_ps = psB.tile([128, 2 * 512], f16, tag="unps")
                for k in range(8):
                    col = k * 128
                    nc.tensor.matmul(un_ps[:, col:col + 128],
                                     scr_t[:, col:col + 128], id_t[:],
                                     is_transpose=True,
                                     start=(k == 0), stop=(k == 7),
                                     skip_group_check=True)
                src = un_ps[:].rearrange("p (ib g t) -> p ib g t",
                                         ib=NIB, g=GPC)
                nc.scalar.copy(mixedv[:, :, :, og * 16:(og + 1) * 16], src)

            def emit_xt(g, xt_map, eng=None):
                eng = eng or nc.sync
                for ib in range(NIB):
                    xt_t = xtpool.tile([128, R], f16, tag=f"xt{ib}",
                                       name=f"xt_g{g}_{ib}")
                    eng.dma_start(
                        out=xt_t[:],
                        in_=x_ext[g * R:(g + 1) * R, ib * 128:(ib + 1) * 128],
                        transpose=True,
                    )
                    xt_map[(g, ib)] = xt_t

            xt_map = {}
            # graph 0's x transposes ride the SCALAR queue pre-mix; the
            # sync queue streams expert weights uninterrupted
            emit_xt(0, xt_map, nc.scalar)
            # first 2 o-groups arrive as ONE contiguous 512KB DMA so the
            # mix has a weight cushion when the cold DMA path delivers
            w4_t = cpool.tile([128, 2 * IN_F], f16, tag="w4")
            nc.sync.dma_start(out=w4_t[:], in_=wp4_ext[:])
            pending = []
            for og in range(NOG):
                if og >= 2:
                    w_t = wpool.tile([128, IN_F], f16, tag="w")
                    nc.sync.dma_start(out=w_t[:], in_=wp_ext[og])
                scr_t = scrpool.tile([128, IN_F], f16, tag="scr")
                for ic in range(2):
                    src_w = (w4_t[:, og * IN_F + ic * 512:
                                  og * IN_F + (ic + 1) * 512]
                             if og < 2 else
                             w_t[:, ic * 512:(ic + 1) * 512])
                    scr_ps = psA.tile([128, 512], f32, tag="scrps")
                    nc.tensor.matmul(scr_ps[:], s1_t[:], src_w,
                                     start=True, stop=True)
                    nc.vector.tensor_copy(
                        scr_t[:, ic * 512:(ic + 1) * 512], scr_ps[:])
                pending.append((og, scr_t))
                if len(pending) > MIX_LAG:
                    emit_unscramble(*pending.pop(0))
            while pending:
                emit_unscramble(*pending.pop(0))

            # ---- MAIN phase ----
            # Output is staged two row-blocks at a time ([128, 2048] fp16 =
            # 512 KB) so stores amortize the per-DMA issue + HBM-receipt cost.
            for g in range(GPC):
                for rb in range(NRB):
                    if rb == 1 and g + 1 < GPC:
                        emit_xt(g + 1, xt_map)
                    if rb % 2 == 0:
                        ostage = opool.tile([128, 2 * OUT_F], f16, tag="osb")
                    for oc in range(2):
                        out_ps = psC.tile([128, 512], f32, tag="outps")
                        for ib in range(NIB):
                            base = ib * (GPC * OUT_F) + g * OUT_F + oc * 512
                            nc.tensor.matmul(
                                out_ps[:],
                                xt_map[(g, ib)][:, rb * 128:(rb + 1) * 128],
                                mixedbuf[:, base:base + 512],
                                start=(ib == 0), stop=(ib == NIB - 1),
                            )
                        col = (rb % 2) * OUT_F + oc * 512
                        nc.vector.tensor_tensor(
                            out=ostage[:, col:col + 512], in0=out_ps[:],
                            in1=breps[g][:, oc * 512:(oc + 1) * 512],
                            op=mybir.AluOpType.add,
                        )
                    if rb % 2 == 1:
                        row0 = g * R + (rb - 1) * 128
                        dst = out_ext[row0:row0 + 256, :].rearrange(
                            "(rb2 p) o -> p rb2 o", p=128)
                        nc.scalar.dma_start(out=dst, in_=ostage[:])
    nc.compile()
    return nc


def _host_prep(x, coefficients, weight_experts, bias_experts, weight_shared,
               bias_shared):
    xb = x.astype(np.float16)
    ident = np.eye(128, dtype=np.float16)
    eye16 = np.eye(16, dtype=np.float32)
    we64 = weight_experts.astype(np.float64)
    wsh64 = weight_shared[0].astype(np.float64)

    in_maps = []
    for c in range(NCORES):
        coef_c = coefficients[c * GPC:(c + 1) * GPC].astype(np.float64)  # [GPC, E]
        # Fold the shared expert into the routed experts: with
        # beta = C^-1 @ 1 (exact for GPC == E graphs per core),
        # sum_e c_ge (W_e + beta_e W_sh) == sum_e c_ge W_e + W_sh.
        beta = np.linalg.solve(coef_c, np.ones(E))
        ve = we64 + beta[:, None, None] * wsh64[None]
        wp = np.ascontiguousarray(
            ve.reshape(E, NOG, 16, IN_F).transpose(1, 0, 2, 3)
            .reshape(NOG, 128, IN_F).astype(np.float16))
        mixed_b = (coef_c @ (bias_experts.astype(np.float64)
                             + beta[:, None] * bias_shared[0].astype(np.float64)))
        brep = np.ascontiguousarray(
            np.broadcast_to(mixed_b[:, None, :], (GPC, 128, OUT_F))
            .reshape(GPC * 128, OUT_F).astype(np.float16))
        s1 = np.kron(coef_c.T, eye16).astype(np.float16)
        wp4 = np.ascontiguousarray(
            wp[0:2].transpose(1, 0, 2).reshape(128, 2 * IN_F))
        in_maps.append({
            "x": np.ascontiguousarray(xb[c * GPC * R:(c + 1) * GPC * R]),
            "wp": wp,
            "wp4": wp4,
            "s1": np.ascontiguousarray(s1),
            "ident": ident,
            "brep": brep,
        })
    return in_maps


def kernel(x, coefficients, weight_experts, bias_experts, weight_shared,
           bias_shared, _want_trace=False):
    if "nc" not in _CACHED:
        _CACHED["nc"] = build_kernel()
    nc = _CACHED["nc"]
    in_maps = _host_prep(x, coefficients, weight_experts, bias_experts,
                         weight_shared, bias_shared)
    kw = {}
    if _want_trace:
        kw = dict(trace=True)
    res = run_bass_kernel_spmd(nc, in_maps, core_ids=list(range(NCORES)), **kw)
    _CACHED["last_result"] = res
    out = np.concatenate(
        [res.results[c]["out"].astype(np.float32) for c in range(NCORES)],
        axis=0)
    return out


# revision 29
# speedup vs baseline: 1.0215x; 1.0215x over previous
"""Trainium2 Bass kernel for nn_MOLELinear (MoE-style mixed linear layer).

Math (per graph g):
    mixed_w[g] = sum_e coefficients[g, e] * weight_experts[e] + weight_shared[0]
    mixed_b[g] = coefficients[g] @ bias_experts + bias_shared[0]
    out[g]     = x[g] @ mixed_w[g].T + mixed_b[g]

Strategy (8 NeuronCores, data-parallel over graphs; 8 graphs per core):
  * Beta fold (host): each core handles exactly E=8 graphs, so
    beta = C_core^-1 @ 1 exists and V_e = W_e + beta_e * W_sh satisfies
    sum_e c_ge V_e == sum_e c_ge W_e + W_sh exactly. The shared expert
    (weights AND bias) thus vanishes from the device kernel.
  * Bias rows are mixed on the host (c_g @ bias'_e, ~1 MFLOP), uploaded
    pre-replicated, and loaded over the idle gpsimd queue; they are only
    consumed by the main-phase PSUM evacuation adds.
  * MIX phase on the PE: one matmul per (o-16-group, i-512-chunk) with a
    block-diagonal coefficient matrix S1[(e,t),(g,t')] = c[g,e]*eye16 computes
    mixed rows for all 8 graphs at once (K=128 fully used; V streamed once).
    The "(g,t)-scrambled" result is unscrambled by 8 PE transpose-mode
    passes packed into a single PSUM bank (start flag only on the first --
    start=True marks the whole 2KB zero-region pending-zero), then one bulk
    strided ACT copy per o-group lands it in [i, (ib,g,o)] layout.
    The mix is software-pipelined (transposes lag the s1 matmuls by
    MIX_LAG o-groups, psA 4 deep) so the PE never stalls on the evacuation.
  * MAIN phase: x is loaded pre-transposed via DMA xbar transpose (fp16),
    double-buffered across graphs (graph 0 prefetched on the scalar queue
    during the mix); out[g] tiles accumulate over 8 i-blocks in PSUM; bias
    is added during the PSUM->SBUF evacuation (DVE tensor_tensor) into
    [128, 2048] staging tiles so stores ship as 512 KB DMAs (amortizes the
    ~0.6us issue + ~2us HBM write-receipt per DMA).
    Output is stored fp16 and widened to fp32 on the host (rel-err budget
    allows it; halves the store traffic and evacuation cost).
  * All matmul operands are fp16 (PSUM accumulation fp32).
"""

import numpy as np

import concourse.bacc as bacc
import concourse.mybir as mybir
import concourse.tile as tile
from concourse.bass_utils import run_bass_kernel_spmd

f32 = mybir.dt.float32
f16 = mybir.dt.float16  # fp16: same PE rate as bf16, 11-bit mantissa

NCORES = 8
G = 64                  # total graphs
GPC = G // NCORES       # graphs per core
R = 1024                # rows per graph
IN_F = 1024
OUT_F = 1024
E = 8                   # routed experts
NOG = OUT_F // 16       # number of 16-row o-groups (64)
NIB = IN_F // 128       # i blocks (8)
NRB = R // 128          # row blocks per graph (8)
MIX_LAG = 2             # o-groups the unscramble lags behind the s1 matmuls

_CACHED = {}


def build_kernel():
    nc = bacc.Bacc(None, target_bir_lowering=False)

    x_ext = nc.declare_dram_parameter("x", [GPC * R, IN_F], f16, isOutput=False)
    wp_ext = nc.declare_dram_parameter("wp", [NOG, 128, IN_F], f16, isOutput=False)
    s1_ext = nc.declare_dram_parameter("s1", [128, 128], f16, isOutput=False)
    id_ext = nc.declare_dram_parameter("ident", [128, 128], f16, isOutput=False)
    brep_ext = nc.declare_dram_parameter("brep", [GPC * 128, OUT_F], f16,
                                         isOutput=False)
    out_ext = nc.declare_dram_parameter("out", [GPC * R, OUT_F], f16,
                                        isOutput=True)

    with tile.TileContext(nc) as tc:
        with (
            tc.tile_pool(name="consts", bufs=1) as cpool,
            tc.tile_pool(name="mixed", bufs=1) as mpool,
            tc.tile_pool(name="breps", bufs=1) as bpool,
            tc.tile_pool(name="xtp", bufs=2) as xtpool,
            tc.tile_pool(name="wstage", bufs=4) as wpool,
            tc.tile_pool(name="scr", bufs=MIX_LAG + 1) as scrpool,
            tc.tile_pool(name="outs", bufs=3) as opool,
            tc.tile_pool(name="psA", bufs=3, space="PSUM") as psA,
            tc.tile_pool(name="psB", bufs=3, space="PSUM") as psB,
            tc.tile_pool(name="psC", bufs=2, space="PSUM") as psC,
        ):
            # ---- constants ----
            s1_t = cpool.tile([128, 128], f16, tag="s1")
            id_t = cpool.tile([128, 128], f16, tag="id")
            nc.sync.dma_start(out=s1_t[:], in_=s1_ext[:])
            nc.sync.dma_start(out=id_t[:], in_=id_ext[:])

            # ---- mixed weights, transposed: [128 i, (ib, g, o)] fp16 ----
            mixedbuf = mpool.tile([128, NIB * GPC * OUT_F], f16, tag="mixed",
                                  name="mixedbuf")
            mixedv = mixedbuf[:].rearrange("p (ib g o) -> p ib g o",
                                           ib=NIB, g=GPC)

            # ---- bias rows: host-mixed and pre-replicated, loaded on the
            # otherwise-idle gpsimd queue (keeps the PE/ACT/sync queues
            # clear so the s1 stream starts as soon as w(0) lands) ----
            breps = []
            for g in range(GPC):
                brep_t = bpool.tile([128, OUT_F], f16, tag=f"brep{g}",
                                    name=f"brep{g}")
                nc.gpsimd.dma_start(out=brep_t[:],
                                    in_=brep_ext[g * 128:(g + 1) * 128, :])
                breps.append(brep_t)

            # ---- MIX phase ----
            def emit_unscramble(og, scr_t):
                # 8 PE transposes into one PSUM bank, then one bulk copy.
                un_ps = psB.tile([128, 2 * 512], f16, tag="unps")
                for k in range(8):
                    col = k * 128
                    nc.tensor.matmul(un_ps[:, col:col + 128],
                                     scr_t[:, col:col + 128], id_t[:],
                                     is_transpose=True,
                                     start=(k == 0), stop=(k == 7),
                                     skip_group_check=True)
                src = un_ps[:].rearrange("p (ib g t) -> p ib g t",
                                         ib=NIB, g=GPC)
                nc.scalar.copy(mixedv[:, :, :, og * 16:(og + 1) * 16], src)

            def emit_xt(g, xt_map, eng=None):
                eng = eng or nc.sync
                for ib in range(NIB):
                    xt_t = xtpool.tile([128, R], f16, tag=f"xt{ib}",
                                       name=f"xt_g{g}_{ib}")
                    eng.dma_start(
                        out=xt_t[:],
                        in_=x_ext[g * R:(g + 1) * R, ib * 128:(ib + 1) * 128],
                        transpose=True,
                    )
                    xt_map[(g, ib)] = xt_t

            xt_map = {}
            # graph 0's x transposes ride the SCALAR queue pre-mix; the
            # sync queue streams expert weights uninterrupted
            emit_xt(0, xt_map, nc.scalar)
            pending = []
            for og in range(NOG):
                w_t = wpool.tile([128, IN_F], f16, tag="w")
                nc.sync.dma_start(out=w_t[:], in_=wp_ext[og])
                scr_t = scrpool.tile([128, IN_F], f16, tag="scr")
                for ic in range(2):
                    scr_ps = psA.tile([128, 512], f32, tag="scrps")
                    nc.tensor.matmul(scr_ps[:], s1_t[:],
                                     w_t[:, ic * 512:(ic + 1) * 512],
                                     start=True, stop=True)
                    nc.vector.tensor_copy(
                        scr_t[:, ic * 512:(ic + 1) * 512], scr_ps[:])
                pending.append((og, scr_t))
                if len(pending) > MIX_LAG:
                    emit_unscramble(*pending.pop(0))
            while pending:
                emit_unscramble(*pending.pop(0))

            # ---- MAIN phase ----
            # Output is staged two row-blocks at a time ([128, 2048] fp16 =
            # 512 KB) so stores amortize the per-DMA issue + HBM-receipt cost.
            for g in range(GPC):
                for rb in range(NRB):
                    if rb == 1 and g + 1 < GPC:
                        emit_xt(g + 1, xt_map)
                    if rb % 2 == 0:
                        ostage = opool.tile([128, 2 * OUT_F], f16, tag="osb")
                    for oc in range(2):
                        out_ps = psC.tile([128, 512], f32, tag="outps")
                        for ib in range(NIB):
                            base = ib * (GPC * OUT_F) + g * OUT_F + oc * 512
                            nc.tensor.matmul(
                                out_ps[:],
                                xt_map[(g, ib)][:, rb * 128:(rb + 1) * 128],
                                mixedbuf[:, base:base + 512],
                                start=(ib == 0), stop=(ib == NIB - 1),
                            )
                        col = (rb % 2) * OUT_F + oc * 512
                        nc.vector.tensor_tensor(
                            out=ostage[:, col:col + 512], in0=out_ps[:],
                            in1=breps[g][:, oc * 512:(oc + 1) * 512],
                            op=mybir.AluOpType.add,
                        )
                    if rb % 2 == 1:
                        row0 = g * R + (rb - 1) * 128
                        dst = out_ext[row0:row0 + 256, :].rearrange(
                            "(rb2 p) o -> p rb2 o", p=128)
                        nc.scalar.dma_start(out=dst, in_=ostage[:])
    nc.compile()
    return nc


def _host_prep(x, coefficients, weight_experts, bias_experts, weight_shared,
               bias_shared):
    xb = x.astype(np.float16)
    ident = np.eye(128, dtype=np.float16)
    eye16 = np.eye(16, dtype=np.float32)
    we64 = weight_experts.astype(np.float64)
    wsh64 = weight_shared[0].astype(np.float64)

    in_maps = []
    for c in range(NCORES):
        coef_c = coefficients[c * GPC:(c + 1) * GPC].astype(np.float64)  # [GPC, E]
        # Fold the shared expert into the routed experts: with
        # beta = C^-1 @ 1 (exact for GPC == E graphs per core),
        # sum_e c_ge (W_e + beta_e W_sh) == sum_e c_ge W_e + W_sh.
        beta = np.linalg.solve(coef_c, np.ones(E))
        ve = we64 + beta[:, None, None] * wsh64[None]
        wp = np.ascontiguousarray(
            ve.reshape(E, NOG, 16, IN_F).transpose(1, 0, 2, 3)
            .reshape(NOG, 128, IN_F).astype(np.float16))
        mixed_b = (coef_c @ (bias_experts.astype(np.float64)
                             + beta[:, None] * bias_shared[0].astype(np.float64)))
        brep = np.ascontiguousarray(
            np.broadcast_to(mixed_b[:, None, :], (GPC, 128, OUT_F))
            .reshape(GPC * 128, OUT_F).astype(np.float16))
        s1 = np.kron(coef_c.T, eye16).astype(np.float16)
        in_maps.append({
            "x": np.ascontiguousarray(xb[c * GPC * R:(c + 1) * GPC * R]),
            "wp": wp,
            "s1": np.ascontiguousarray(s1),
            "ident": ident,
            "brep": brep,
        })
    return in_maps


def kernel(x, coefficients, weight_experts, bias_experts, weight_shared,
           bias_shared, _want_trace=False):
    if "nc" not in _CACHED:
        _CACHED["nc"] = build_kernel()
    nc = _CACHED["nc"]
    in_maps = _host_prep(x, coefficients, weight_experts, bias_experts,
                         weight_shared, bias_shared)
    kw = {}
    if _want_trace:
        kw = dict(trace=True)
    res = run_bass_kernel_spmd(nc, in_maps, core_ids=list(range(NCORES)), **kw)
    _CACHED["last_result"] = res
    out = np.concatenate(
        [res.results[c]["out"].astype(np.float32) for c in range(NCORES)],
        axis=0)
    return out
